# revision 91
# baseline (speedup 1.0000x reference)
"""Trainium2 Bass kernel for nn_CrossFusion — polynomial-softmax rewrite.

k_dim = 1 makes the attention scores rank-1: e[s,t] = exp(q_s*k_t), so
    den(q) = sum_m q^m/m! * S_m,   S_m = sum_t k_t^m
    num(q) = sum_m q^m/m! * T_m,   T_m = sum_t v_t k_t^m
With |q*k| <= ~1.2 a low-degree truncation suffices (the num/den truncation
errors cancel in the softmax ratio); the whole [S1,S2] attention collapses to
power sums over t plus a per-s Horner evaluation.

x2 path: load f32 natural halves -> convert bf16 + square -> 64 PE
transposes build a stacked T-form tile xs2 = [x2^T ; (x2^2)^T] ([128, 4096]:
partitions 0:64 hold x2 features, 64:128 the squares; column i <-> row
t = 32p' + cc).  All projections (k0, k1, v-num0, v-num1, nv2, n2) are then
ONE bf16 PE matmul per 512-column chunk with the CBN affine folded into the
weight columns:
    v2.Wv = x2.(A*Wv) + B.Wv ;  ||v2||^2 = x2^2.A^2 + x2.(2AB) + ||B||^2
Each PSUM bank is copied out whole and de-transposed back to column form
with 8 PE transposes; the power-sum ops read that PSUM directly and emit
their per-partition partial sums via accum_out (no separate reduces), then
one all-ones matmul broadcasts the cross-partition totals.
rsqrt = ACT Sqrt + DVE reciprocal; final sigmoid is a single ACT op.
x1 (query side) is transposed ON-CHIP with 8 PE transposes of the bf16
slab + its squares; one stacked matmul gives q0,q1,||q||^2 per s.
All small weights arrive host-concatenated in one DMA (wcat).
Output rows are s = 4p + c (p partition, c in 0..3).
"""
import numpy as np

S = 4096
D = 64
H = 2
NCORES = 8
SSH = S // NCORES   # 512 query rows per core
M = 4               # Taylor degree
EPS_BN = 1e-5

_CACHE = {}


def _build(split=True):
    import concourse.bass as bass
    import concourse.tile as tile
    import concourse.mybir as mybir
    from concourse.masks import make_identity

    f32 = mybir.dt.float32
    bf16 = mybir.dt.bfloat16
    AF = mybir.ActivationFunctionType
    ALU = mybir.AluOpType
    P = 128

    nc = bass.Bass("TRN2", target_bir_lowering=False, debug=False)

    x1s = nc.dram_tensor("x1s", [SSH, D], bf16, kind="ExternalInput")
    x1f = nc.dram_tensor("x1f", [S, D], bf16, kind="ExternalInput")
    x2 = nc.dram_tensor("x2", [S, D], bf16, kind="ExternalInput")
    wcat = nc.dram_tensor("wcat", [D, 270], f32, kind="ExternalInput")
    y = nc.dram_tensor("y", [SSH, 2], f32, kind="ExternalOutput")

    with tile.TileContext(nc) as tc:
        with tc.tile_pool(name="sb", bufs=1) as sb, \
             tc.tile_pool(name="psum", bufs=1, space="PSUM") as psum:

            # ---------------- PSUM (2KB banks) ------------------------------
            TR1 = psum.tile([P, 8 * P], bf16, name="TR1")
            TR2 = psum.tile([P, 8 * P], bf16, name="TR2")
            PP1 = psum.tile([P, 512], f32, name="PP1")
            PP2 = psum.tile([P, 512], f32, name="PP2")
            TP = psum.tile([P, 8 * P], bf16, name="TPn")
            TQT2m = psum.tile([P, 528], bf16, name="TQT2")
            TQT2 = TQT2m[:, 0:512]
            TQ = TQT2m[:, 512:528]
            PQ3 = psum.tile([4, 512], f32, name="PQ3")
            SM = psum.tile([P, 160], f32, name="SM")
            h_ps = SM[0:64, 0:1]
            zg_ps = SM[0:64, 1:2]
            zb_ps = SM[0:64, 2:3]
            dg_ps = SM[:, 3:4]
            db_ps = SM[:, 4:5]
            mu_ps = SM[0:64, 5:6]
            msq_ps = SM[0:64, 6:7]
            mu128_ps = SM[:, 7:8]
            msq128_ps = SM[:, 8:9]
            consts_ps = SM[0:1, 9:12]
            cb9_ps = SM[:, 64:73]
            cb52_ps = SM[:, 73:73 + 4 * (M + 1)]

            # ---------------- SBUF ------------------------------------------
            xpl_a = sb.tile([P, 16 * D], bf16)     # x2 naturals (bf16, host)
            xpl_b = sb.tile([P, 16 * D], bf16)
            xi_a = sb.tile([P, 32 * D], bf16)   # interleaved [x2|x2^2] pairs
            xi_b = sb.tile([P, 32 * D], bf16)
            xs2 = sb.tile([P, S], bf16)            # stacked T-form [x2T; x2sqT]
            x1sbn = sb.tile([P, SSH // 2], bf16)
            x1sqn = sb.tile([P, SSH // 2], bf16)
            x1fbig = sb.tile([P, (S // P) * D], bf16)
            wcat_sb = sb.tile([D, 270], f32)
            lhsT1 = sb.tile([P, 6], bf16)
            lhsTq = sb.tile([P, 4], bf16)
            ident = sb.tile([P, P], bf16)
            identf = sb.tile([D, D], f32)
            ones_col = sb.tile([P, 1], f32)
            ones128 = sb.tile([P, P], f32)
            ones_bf = sb.tile([P, 1], bf16)
            ones_row = sb.tile([1, P], f32)
            eps_col = sb.tile([P, 1], f32)
            mu_sb = sb.tile([D, 1], f32)
            msq_sb = sb.tile([D, 1], f32)
            musq = sb.tile([P, 1], f32)
            mu128s = sb.tile([P, 1], f32)
            var128 = sb.tile([P, 1], f32)
            sq128 = sb.tile([P, 1], f32)
            rs128 = sb.tile([P, 1], f32)
            A128 = sb.tile([P, 1], f32)
            muA = sb.tile([P, 1], f32)
            B128 = sb.tile([P, 1], f32)
            zg_sb = sb.tile([D, 1], f32)
            zb_sb = sb.tile([D, 1], f32)
            h_col = sb.tile([D, 1], f32)
            crow = sb.tile([1, 9], f32)
            constsb = sb.tile([P, 9], f32)
            PPs = sb.tile([P, 1024], bf16)
            TQTs = sb.tile([P, 512], bf16)
            PQs3 = sb.tile([4, 512], bf16)
            rsnn = sb.tile([P, 64], f32)
            stnn = sb.tile([P, 64], f32)
            rsq1 = sb.tile([P, 4], f32)
            st1 = sb.tile([P, 4], f32)
            K_all = sb.tile([P, (M + 1) * 2 * 64], bf16)
            R = sb.tile([P, 4 * (M + 1) + 4], f32)
            cbrow = sb.tile([P, 4 * (M + 1)], f32)
            qhat = sb.tile([P, 8], f32)
            Qs = sb.tile([P, 8], f32)
            eD = sb.tile([P, 8], f32)
            oD = sb.tile([P, 8], f32)
            eN = sb.tile([P, 8], f32)
            oN = sb.tile([P, 8], f32)
            rden = sb.tile([P, 8], f32)
            rr = sb.tile([P, 8], f32)
            zt = sb.tile([P, 8], f32)
            sig = sb.tile([P, 8], f32)

            x2r = x2.rearrange("(p cc) d -> p (cc d)", p=P)
            x1sr = x1s.rearrange("(p cc) d -> p (cc d)", p=P)
            x1fr = x1f.rearrange("(p c) d -> p (c d)", p=P)

            # ===== loads, ordered by when each gating chain needs them ======
            # x2 first (it gates the longest chain: convert/square/transpose/
            # copies); x1f second-to-last (its h->MLP->dg chain is short);
            # x1s last (q path has the most slack).
            nc.sync.dma_start(xpl_a[:], x2r[:, 0:16 * D])
            nc.sync.dma_start(xpl_b[:, 0:8 * D], x2r[:, 16 * D:24 * D])
            nc.sync.dma_start(xpl_b[:, 8 * D:16 * D], x2r[:, 24 * D:32 * D])
            nc.sync.dma_start(wcat_sb[:], wcat[:, :])
            nc.sync.dma_start(x1fbig[:, 0:16 * D], x1fr[:, 0:16 * D])
            nc.sync.dma_start(x1fbig[:, 16 * D:32 * D], x1fr[:, 16 * D:32 * D])
            nc.sync.dma_start(x1sbn[:], x1sr[:, :])

            # static prep
            nc.vector.memset(ones_col[:], 1.0)
            nc.vector.memset(ones128[:], 1.0)
            nc.vector.memset(ones_bf[:], 1.0)
            nc.vector.memset(ones_row[:], 1.0)
            nc.vector.memset(eps_col[:], EPS_BN)
            make_identity(nc, ident[:])
            make_identity(nc, identf[:])
            nc.gpsimd.memset(lhsTq[:], 0.0)
            nc.gpsimd.memset(lhsT1[:], 0.0)
            nc.gpsimd.memset(lhsTq[64:128, 2:3], 1.0)
            nc.gpsimd.memset(lhsT1[64:128, 5:6], 1.0)

            # small-weight casts from the concatenated block (f32 -> bf16)
            nc.vector.tensor_copy(lhsTq[0:64, 0:2], wcat_sb[:, 256:258])  # Wq
            nc.vector.tensor_copy(lhsT1[0:64, 0:2], wcat_sb[:, 258:260])  # Wk
            wvv = wcat_sb[:, 260:262]
            nc.gpsimd.tensor_copy(crow[:, 3:7], wcat_sb[0:1, 262:266])    # Wo
            nc.gpsimd.tensor_copy(crow[:, 7:9], wcat_sb[0:1, 266:268])    # bo

            def kslice(m):
                return K_all[:, (2 * m) * 64:(2 * m + 1) * 64]

            def uslice(m):
                return K_all[:, (2 * m + 1) * 64:(2 * m + 2) * 64]

            # S_0 per-partition partial = 32 (handled as a constant in R)
            nc.vector.memset(R[:, 0:2], 32.0)

            # ============ x2 converts + squares + transposes ================
            # quarter q covers natural chunks cc in [8q, 8q+8); transpose of
            # chunk cc lands at TR[0:64 | 64:128, 128j:128(j+1)], j = cc%8;
            # xs2 column i = 1024q + 128j + p' maps to t = 32p' + cc.
            # Converts (ACT for b) and squares (DVE, straight from the f32
            # naturals) run in parallel; xs2 copy-outs are emitted later in
            # per-engine readiness order.
            nc.vector.memset(PP1[:], 0.0)
            nc.vector.memset(PP2[:], 0.0)
            with tc.high_priority():
                # interleave the host-cast bf16 naturals with their squares
                # (plain copy on ACT, squares on DVE, in parallel)
                xav = xi_a[:].rearrange("p (c two d) -> p c two d", two=2, d=D)
                xbv = xi_b[:].rearrange("p (c two d) -> p c two d", two=2, d=D)
                xanat = xpl_a[:].rearrange("p (c d) -> p c d", d=D)
                xbnat = xpl_b[:].rearrange("p (c d) -> p c d", d=D)
                nc.scalar.copy(xav[:, :, 0, :], xanat)
                nc.vector.tensor_tensor(out=xav[:, :, 1, :], in0=xanat,
                                        in1=xanat, op=ALU.mult)
                for qq in range(2):
                    ql = slice(8 * qq, 8 * qq + 8)
                    nc.scalar.copy(xbv[:, ql, 0, :], xbnat[:, ql, :])
                    nc.vector.tensor_tensor(out=xbv[:, ql, 1, :],
                                            in0=xbnat[:, ql, :],
                                            in1=xbnat[:, ql, :], op=ALU.mult)
                for q in range(4):
                    TR = (TR1, TR2, TR1, TR2)[q]
                    xi = (xi_a, xi_b)[q // 2]
                    for j in range(8):
                        cl = slice((8 * (q % 2) + j) * 2 * D, (8 * (q % 2) + j + 1) * 2 * D)
                        nc.tensor.transpose(TR[:, 128 * j:128 * (j + 1)], xi[:, cl], ident[:])
                    if q == 2:
                        nc.scalar.copy(xs2[:, 1024 * q:1024 * (q + 1)], TR[:])
                    else:
                        nc.vector.tensor_copy(xs2[:, 1024 * q:1024 * (q + 1)], TR[:])
                # per-half stats (ap-1 matmuls are ~free on PE)
                for half, xn in enumerate((xpl_a, xpl_b)):
                    for cc in range(16):
                        nc.tensor.matmul(mu_ps, xn[:, cc * D:(cc + 1) * D], ones_bf[:],
                                         start=(half == 0 and cc == 0), stop=(half == 1 and cc == 15))
                for half, xi in enumerate((xi_a, xi_b)):
                    for cc in range(16):
                        nc.tensor.matmul(msq_ps, xi[:, (2 * cc + 1) * D:(2 * cc + 2) * D], ones_bf[:],
                                         start=(half == 0 and cc == 0), stop=(half == 1 and cc == 15))
                TCf = S // P
                for c in range(TCf):
                    nc.tensor.matmul(h_ps, x1fbig[:, c * D:(c + 1) * D], ones_bf[:],
                                     start=(c == 0), stop=(c == TCf - 1))

            # ============ h epilogue + CBN MLPs (ready before b lands) ======
            nc.scalar.activation(h_col[:], h_ps, AF.Copy, scale=1.0 / S)
            nc.tensor.matmul(zg_ps, wcat_sb[:, 0:64], h_col[:], start=True, stop=True)
            nc.tensor.matmul(zb_ps, wcat_sb[:, 128:192], h_col[:], start=True, stop=True)
            nc.scalar.activation(zg_sb[:], zg_ps, AF.Relu)
            nc.scalar.activation(zb_sb[:], zb_ps, AF.Relu)
            nc.tensor.matmul(dg_ps[0:64, :], wcat_sb[:, 64:128], zg_sb[:], start=True, stop=True)
            nc.tensor.matmul(db_ps[0:64, :], wcat_sb[:, 192:256], zb_sb[:], start=True, stop=True)

            # ============ x1 query path (on-chip transposes) ================
            # slab row s = 4p + cc ; TQT2 col j = 128c + p' <-> s = 4p' + c;
            # partitions 0:64 features, 64:128 squares.
            nc.gpsimd.tensor_tensor(out=x1sqn[:], in0=x1sbn[:], in1=x1sbn[:],
                                    op=ALU.mult)
            for c in range(4):
                nc.tensor.transpose(TQT2[0:64, 128 * c:128 * (c + 1)],
                                    x1sbn[:, 64 * c:64 * (c + 1)], ident[:])
                nc.tensor.transpose(TQT2[64:128, 128 * c:128 * (c + 1)],
                                    x1sqn[:, 64 * c:64 * (c + 1)], ident[:])

            # ============ x2 stats epilogue + A, B, lhsT columns ============
            # single-hop chain on 64 partitions, reading the stat PSUM cells
            # directly (1/S factors folded into the op scalars); only the
            # sqrt visits ACT, and only the A^2 column needs an upper-half
            # duplicate (one identity matmul).
            Aup_ps = mu128_ps[64:128, :]
            nc.scalar.activation(musq[0:64, :], mu_ps, AF.Square, scale=1.0 / S)
            nc.vector.scalar_tensor_tensor(out=var128[0:64, :], in0=msq_ps,
                                           scalar=1.0 / S, in1=musq[0:64, :],
                                           op0=ALU.mult, op1=ALU.subtract)
            nc.scalar.activation(sq128[0:64, :], var128[0:64, :], AF.Sqrt,
                                 bias=eps_col[0:64, :])
            nc.vector.reciprocal(rs128[0:64, :], sq128[0:64, :])
            nc.vector.scalar_tensor_tensor(out=A128[0:64, :], in0=dg_ps[0:64, :],
                                           scalar=1.0, in1=rs128[0:64, :],
                                           op0=ALU.add, op1=ALU.mult)
            nc.vector.scalar_tensor_tensor(out=muA[0:64, :], in0=mu_ps,
                                           scalar=1.0 / S, in1=A128[0:64, :],
                                           op0=ALU.mult, op1=ALU.mult)
            nc.vector.tensor_tensor(out=B128[0:64, :], in0=db_ps[0:64, :],
                                    in1=muA[0:64, :], op=ALU.subtract)
            for hh in range(H):
                nc.vector.tensor_tensor(out=lhsT1[0:64, 2 + hh:3 + hh],
                                        in0=A128[0:64, :], in1=wvv[:, hh:hh + 1], op=ALU.mult)
            nc.vector.scalar_tensor_tensor(out=lhsT1[0:64, 4:5], in0=A128[0:64, :], scalar=2.0,
                                           in1=B128[0:64, :], op0=ALU.mult, op1=ALU.mult)
            nc.tensor.matmul(Aup_ps, identf[:], A128[0:64, :], start=True, stop=True)
            nc.scalar.activation(lhsT1[64:128, 4:5], Aup_ps, AF.Square)

            nc.tensor.matmul(consts_ps[:, 0:2], B128[0:64, :], wvv[:, :], start=True, stop=True)
            nc.tensor.matmul(consts_ps[:, 2:3], B128[0:64, :], B128[0:64, :], start=True, stop=True)
            nc.scalar.copy(crow[:, 0:3], consts_ps)
            nc.tensor.matmul(cb9_ps, ones_row[:], crow[:], start=True, stop=True)
            nc.scalar.copy(constsb[:], cb9_ps)

            # q-projection: one stacked matmul gives (q0, q1, ||q||^2) per s
            # (held out of the heap until the lhsT1 chain has cleared ACT/DVE)
            with tc.tile_wait_until(0.0118):
                nc.scalar.copy(TQTs[:], TQT2[:])
                nc.tensor.matmul(PQ3[0:3, :], lhsTq[:, 0:3], TQTs[:],
                                 start=True, stop=True)
                nc.scalar.copy(PQs3[0:3, :], PQ3[0:3, :])
                for b in range(4):
                    nc.tensor.transpose(TQ[:, 4 * b:4 * b + 3],
                                        PQs3[0:3, 128 * b:128 * (b + 1)],
                                        ident[0:3, 0:3])
            TQv = TQ[:].rearrange("p (b q) -> p b q", q=4)

            # ============ stacked projections ===============================
            # chunk c = 4T + u -> rows 32u..32u+6 of PP{T+1}; after the 4th
            # chunk each bank is copied out whole (rows 6..31 of each 32-row
            # group are pre-zeroed by the memsets above).
            CW = 512
            for c in range(8):
                cs = slice(c * CW, (c + 1) * CW)
                pp = (PP1, PP2)[c // 4]
                u = c % 4
                nc.tensor.matmul(pp[32 * u:32 * u + 6, :], lhsT1[:], xs2[:, cs],
                                 start=True, stop=True, tile_position=(0, 32 * u))
            nc.vector.tensor_copy(PPs[:, 0:512], PP1[:])
            nc.scalar.copy(PPs[:, 512:1024], PP2[:])

            # ============ de-transpose (downstream reads PSUM directly) =====
            for g in range(8):
                nc.tensor.transpose(TP[:, 128 * g:128 * (g + 1)],
                                    PPs[:, 128 * g:128 * (g + 1)], ident[:])
            Cv = TP[:].rearrange("p (g u q) -> p g u q", g=8, u=4)

            # ==== rsqrt = reciprocal(sqrt(x)): ACT sqrt + DVE recip; n2 first
            # (khat needs only rsn2 — keep its chain free of the nv2 sqrt) ==
            stnnv = stnn[:].rearrange("p (g u e) -> p g u e", g=8, u=4)
            rsnnv = rsnn[:].rearrange("p (g u e) -> p g u e", g=8, u=4)
            nc.scalar.activation(stnnv[:, :, :, 1:2], Cv[:, :, :, 5:6], AF.Sqrt)
            nc.vector.reciprocal(rsnnv[:, :, :, 1:2], stnnv[:, :, :, 1:2])
            nc.scalar.activation(stnnv[:, :, :, 0:1], Cv[:, :, :, 4:5], AF.Sqrt,
                                 bias=constsb[:, 2:3])
            nc.vector.reciprocal(rsnnv[:, :, :, 0:1], stnnv[:, :, :, 0:1])
            nc.scalar.activation(st1[:].rearrange("p (b o) -> p b o", o=1),
                                 TQv[:, :, 2:3], AF.Sqrt)
            nc.vector.reciprocal(rsq1[:], st1[:])

            # ============ k^, v^, q^, fused power sums ======================
            # per-head 32-col slices; accum_out collects the per-partition
            # partial power sums directly into R (no trailing reduces)
            khat = kslice(1)
            vhat = uslice(0)
            rsnnv = rsnn[:].rearrange("p (g u e) -> p g u e", g=8, u=4)
            rsn2v = rsnnv[:, :, :, 1:2]
            rsnvv = rsnnv[:, :, :, 0:1]

            def hsl(sl, hh):
                return sl[:, 32 * hh:32 * (hh + 1)].rearrange(
                    "p (g u o) -> p g u o", g=8, o=1)

            # khat + k-power tree on DVE (kp3, kp4 both branch off kp2; the
            # 1/m! factors are folded into the tree scalars); vhat + u1..u3
            # products on Pool; u4 on DVE right after kp4.
            for hh in range(H):
                nc.vector.scalar_tensor_tensor(
                    out=hsl(khat, hh), in0=Cv[:, :, :, hh:hh + 1], scalar=1.0,
                    in1=rsn2v, op0=ALU.mult, op1=ALU.mult,
                    accum_out=R[:, 4 + hh:5 + hh])
            for hh in range(H):  # vhat reads PSUM -> must be DVE, not Pool
                nc.vector.scalar_tensor_tensor(
                    out=hsl(vhat, hh),
                    in0=Cv[:, :, :, 2 + hh:3 + hh], scalar=constsb[:, hh:hh + 1],
                    in1=rsnvv, op0=ALU.add, op1=ALU.mult,
                    accum_out=R[:, 2 + hh:3 + hh])
            for hh in range(H):  # kp2 = khat^2 / 2
                nc.vector.scalar_tensor_tensor(
                    out=hsl(kslice(2), hh), in0=hsl(khat, hh),
                    scalar=0.5, in1=hsl(khat, hh),
                    op0=ALU.mult, op1=ALU.mult,
                    accum_out=R[:, 8 + hh:9 + hh])
            for hh in range(H):  # kp3 = kp2 * khat / 3
                nc.vector.scalar_tensor_tensor(
                    out=hsl(kslice(3), hh), in0=hsl(khat, hh),
                    scalar=1.0 / 3.0, in1=hsl(kslice(2), hh),
                    op0=ALU.mult, op1=ALU.mult,
                    accum_out=R[:, 12 + hh:13 + hh])
            for hh in range(H):  # kp4 = kp2^2 / 6
                nc.vector.scalar_tensor_tensor(
                    out=hsl(kslice(4), hh), in0=hsl(kslice(2), hh),
                    scalar=1.0 / 6.0, in1=hsl(kslice(2), hh),
                    op0=ALU.mult, op1=ALU.mult,
                    accum_out=R[:, 16 + hh:17 + hh])
            # u-products on Pool (its ISA has no TensorScalarPtr/accum);
            # one strided DVE reduce collects all eight partial sums
            for m in range(1, M + 1):
                for hh in range(H):
                    nc.gpsimd.tensor_tensor(
                        out=hsl(uslice(m), hh), in0=hsl(kslice(m), hh),
                        in1=hsl(vhat, hh), op=ALU.mult)
            Kv = K_all[:].rearrange("p (m two h x) -> p m two h x",
                                    two=2, h=2, x=32)
            Ruv = R[:, 6:22].rearrange("p (m o q) -> p m o q", m=4, o=1)
            nc.vector.reduce_sum(Ruv[:, :, :, 0:2], Kv[:, 1:5, 1:2, :, :],
                                 axis=mybir.AxisListType.X)
            rsq1v = rsq1[:].rearrange("p (b o) -> p b o", o=1)
            for hh in range(H):
                nc.vector.tensor_tensor(
                    out=qhat[:, 4 * hh:4 * (hh + 1)].rearrange("p (b o) -> p b o", o=1),
                    in0=TQv[:, :, hh:hh + 1], in1=rsq1v, op=ALU.mult)
            # all-ones lhsT: one matmul = column sums replicated on all partitions
            nc.tensor.matmul(cb52_ps, ones128[:], R[:], start=True, stop=True)
            nc.vector.tensor_copy(cbrow[:], cb52_ps)

            # ===== Horner, even/odd split (den on DVE, num on Pool) =========
            # p(q) = E(Q) + q*O(Q), Q=q^2; E=c0+c2*Q+c4*Q^2, O=c1+c3*Q
            qx = qhat[:].rearrange("p (h c) -> p h c", h=2)
            Qv = Qs[:].rearrange("p (h c) -> p h c", h=2)

            def cb(i):
                return (cbrow[:, i:i + 2]
                        .rearrange("p (h o) -> p h o", h=2).to_broadcast((P, 2, 4)))

            def cbden(m):
                return cb(4 * m)

            def cbnum(m):
                return cb(4 * m + 2)

            eDv = eD[:].rearrange("p (h c) -> p h c", h=2)
            oDv = oD[:].rearrange("p (h c) -> p h c", h=2)
            eNv = eN[:].rearrange("p (h c) -> p h c", h=2)
            oNv = oN[:].rearrange("p (h c) -> p h c", h=2)
            nc.vector.tensor_tensor(out=Qv, in0=qx, in1=qx, op=ALU.mult)
            # den (DVE)
            nc.vector.tensor_tensor(out=eDv, in0=Qv, in1=cbden(4), op=ALU.mult)
            nc.vector.tensor_tensor(out=oDv, in0=Qv, in1=cbden(3), op=ALU.mult)
            nc.vector.tensor_tensor(out=eDv, in0=eDv, in1=cbden(2), op=ALU.add)
            nc.vector.tensor_tensor(out=oDv, in0=oDv, in1=cbden(1), op=ALU.add)
            nc.vector.tensor_tensor(out=eDv, in0=eDv, in1=Qv, op=ALU.mult)
            nc.vector.tensor_tensor(out=oDv, in0=oDv, in1=qx, op=ALU.mult)
            nc.vector.tensor_tensor(out=eDv, in0=eDv, in1=cbden(0), op=ALU.add)
            nc.vector.tensor_tensor(out=eDv, in0=eDv, in1=oDv, op=ALU.add)
            # num (Pool)
            nc.gpsimd.tensor_tensor(out=eNv, in0=Qv, in1=cbnum(4), op=ALU.mult)
            nc.gpsimd.tensor_tensor(out=oNv, in0=Qv, in1=cbnum(3), op=ALU.mult)
            nc.gpsimd.tensor_tensor(out=eNv, in0=eNv, in1=cbnum(2), op=ALU.add)
            nc.gpsimd.tensor_tensor(out=oNv, in0=oNv, in1=cbnum(1), op=ALU.add)
            nc.gpsimd.tensor_tensor(out=eNv, in0=eNv, in1=Qv, op=ALU.mult)
            nc.gpsimd.tensor_tensor(out=oNv, in0=oNv, in1=qx, op=ALU.mult)
            nc.gpsimd.tensor_tensor(out=eNv, in0=eNv, in1=cbnum(0), op=ALU.add)
            nc.gpsimd.tensor_tensor(out=eNv, in0=eNv, in1=oNv, op=ALU.add)

            # ============ epilogue =========================================
            nc.vector.reciprocal(rden[:], eD[:])
            nc.vector.tensor_tensor(out=rr[:], in0=eN[:], in1=rden[:], op=ALU.mult)
            r0 = rr[:, 0:4].rearrange("p (c o) -> p c o", o=1)
            r1 = rr[:, 4:8].rearrange("p (c o) -> p c o", o=1)
            Zv = zt[:].rearrange("p (c j) -> p c j", j=2)
            nc.vector.tensor_scalar(out=Zv[:, :, 0:1], in0=r0,
                                    scalar1=constsb[:, 3:4], scalar2=constsb[:, 7:8],
                                    op0=ALU.mult, op1=ALU.add)
            nc.vector.tensor_scalar(out=Zv[:, :, 1:2], in0=r0,
                                    scalar1=constsb[:, 4:5], scalar2=constsb[:, 8:9],
                                    op0=ALU.mult, op1=ALU.add)
            nc.vector.scalar_tensor_tensor(out=Zv[:, :, 0:1], in0=r1,
                                           scalar=constsb[:, 5:6],
                                           in1=Zv[:, :, 0:1],
                                           op0=ALU.mult, op1=ALU.add)
            nc.vector.scalar_tensor_tensor(out=Zv[:, :, 1:2], in0=r1,
                                           scalar=constsb[:, 6:7],
                                           in1=Zv[:, :, 1:2],
                                           op0=ALU.mult, op1=ALU.add)
            nc.scalar.activation(sig[:], zt[:], AF.Sigmoid)
            nc.sync.dma_start(y.rearrange("(p c) j -> p (c j)", p=P), sig[:])

    if split:
        _split_waits(nc, mybir)
    return nc


def _split_waits(nc, mybir, maxw=1):
    """This container's walrus build rejects instructions carrying more than
    ~2 sync-wait commands. Split excess waits onto zero-register-write nops
    inserted just before the instruction on the same engine (same-engine
    program order preserves the wait-before-execute semantics)."""
    ctr = 0
    for bb in nc.m.functions[0].blocks:
        new = []
        for inst in bb.instructions:
            si = inst.sync_info
            if si is not None and si.on_wait and len(si.on_wait) > maxw:
                waits = list(si.on_wait)
                ename = str(inst.engine).split(".")[-1]
                for w in waits[:-maxw]:
                    ctr += 1
                    new.append(mybir.InstRegisterMove(
                        name=f"WS-{ctr}",
                        ins=[mybir.ImmediateValue(kind="imm_value", dtype=mybir.dt.int32, value=0)],
                        outs=[mybir.RegisterAccess(kind="register_access", regref=f"{ename}_zero", dtype=mybir.dt.int32)],
                        engine=inst.engine,
                        sync_info=mybir.SyncInfo(on_wait=[w], on_update=[]),
                    ))
                si.on_wait = waits[-maxw:]
            new.append(inst)
        bb.instructions = new
    return nc


def _get_program():
    if "nc" not in _CACHE:
        _CACHE["nc"] = _build()
    return _CACHE["nc"]


def kernel(x1, x2, Wq, Wk, Wv, Wo, bo, Wg1, Wg2, Wb1, Wb2):
    import ml_dtypes
    from concourse import bass_utils

    nc = _get_program()
    bft = ml_dtypes.bfloat16
    x1s_full = np.ascontiguousarray(x1[0]).astype(bft)  # [4096, 64]
    x2s = np.ascontiguousarray(x2[0]).astype(bft)

    wcat = np.zeros((D, 270), dtype=np.float32)
    wcat[:, 0:64] = Wg1
    wcat[:, 64:128] = Wg2
    wcat[:, 128:192] = Wb1
    wcat[:, 192:256] = Wb2
    wcat[:, 256:258] = Wq
    wcat[:, 258:260] = Wk
    wcat[:, 260:262] = Wv
    wcat[0, 262:266] = Wo.reshape(-1)
    wcat[0, 266:268] = bo

    in_maps = []
    for i in range(NCORES):
        in_maps.append({
            "x1s": np.ascontiguousarray(x1s_full[i * SSH:(i + 1) * SSH]),
            "x1f": x1s_full,
            "x2": x2s,
            "wcat": wcat,
        })

    # First execution of a freshly-compiled NEFF occasionally reports a
    # transient device error through the PJRT proxy; a retry succeeds.
    last_err = None
    for attempt in range(3):
        try:
            res = bass_utils.run_bass_kernel_spmd(nc, in_maps, core_ids=list(range(NCORES)))
            out = np.concatenate([res.results[i]["y"] for i in range(NCORES)], axis=0)
            return out.reshape(1, S, 2)
        except Exception as e:  # noqa: BLE001
            last_err = e
            import time
            time.sleep(5)
    raise last_err


# revision 92
# speedup vs baseline: 1.0248x; 1.0248x over previous
"""Trainium2 Bass kernel for nn_CrossFusion — polynomial-softmax rewrite.

k_dim = 1 makes the attention scores rank-1: e[s,t] = exp(q_s*k_t), so
    den(q) = sum_m q^m/m! * S_m,   S_m = sum_t k_t^m
    num(q) = sum_m q^m/m! * T_m,   T_m = sum_t v_t k_t^m
With |q*k| <= ~1.2 a low-degree truncation suffices (the num/den truncation
errors cancel in the softmax ratio); the whole [S1,S2] attention collapses to
power sums over t plus a per-s Horner evaluation.

x2 path: load f32 natural halves -> convert bf16 + square -> 64 PE
transposes build a stacked T-form tile xs2 = [x2^T ; (x2^2)^T] ([128, 4096]:
partitions 0:64 hold x2 features, 64:128 the squares; column i <-> row
t = 32p' + cc).  All projections (k0, k1, v-num0, v-num1, nv2, n2) are then
ONE bf16 PE matmul per 512-column chunk with the CBN affine folded into the
weight columns:
    v2.Wv = x2.(A*Wv) + B.Wv ;  ||v2||^2 = x2^2.A^2 + x2.(2AB) + ||B||^2
Each PSUM bank is copied out whole and de-transposed back to column form
with 8 PE transposes; the power-sum ops read that PSUM directly and emit
their per-partition partial sums via accum_out (no separate reduces), then
one all-ones matmul broadcasts the cross-partition totals.
rsqrt = ACT Sqrt + DVE reciprocal; final sigmoid is a single ACT op.
x1 (query side) is transposed ON-CHIP with 8 PE transposes of the bf16
slab + its squares; one stacked matmul gives q0,q1,||q||^2 per s.
All small weights arrive host-concatenated in one DMA (wcat).
Output rows are s = 4p + c (p partition, c in 0..3).
"""
import numpy as np

S = 4096
D = 64
H = 2
NCORES = 8
SSH = S // NCORES   # 512 query rows per core
M = 4               # Taylor degree
EPS_BN = 1e-5

_CACHE = {}


def _build(split=True):
    import concourse.bass as bass
    import concourse.tile as tile
    import concourse.mybir as mybir
    from concourse.masks import make_identity

    f32 = mybir.dt.float32
    bf16 = mybir.dt.bfloat16
    AF = mybir.ActivationFunctionType
    ALU = mybir.AluOpType
    P = 128

    nc = bass.Bass("TRN2", target_bir_lowering=False, debug=False)

    x1s = nc.dram_tensor("x1s", [SSH, D], bf16, kind="ExternalInput")
    x1f = nc.dram_tensor("x1f", [S, D], bf16, kind="ExternalInput")
    x2 = nc.dram_tensor("x2", [S, D], bf16, kind="ExternalInput")
    wcat = nc.dram_tensor("wcat", [D, 270], f32, kind="ExternalInput")
    y = nc.dram_tensor("y", [SSH, 2], f32, kind="ExternalOutput")

    with tile.TileContext(nc) as tc:
        with tc.tile_pool(name="sb", bufs=1) as sb, \
             tc.tile_pool(name="psum", bufs=1, space="PSUM") as psum:

            # ---------------- PSUM (2KB banks) ------------------------------
            TR1 = psum.tile([P, 8 * P], bf16, name="TR1")
            TR2 = psum.tile([P, 8 * P], bf16, name="TR2")
            PP1 = psum.tile([P, 512], f32, name="PP1")
            PP2 = psum.tile([P, 512], f32, name="PP2")
            TP = psum.tile([P, 8 * P], bf16, name="TPn")
            TQT2m = psum.tile([P, 528], bf16, name="TQT2")
            TQT2 = TQT2m[:, 0:512]
            TQ = TQT2m[:, 512:528]
            PQ3 = psum.tile([4, 512], f32, name="PQ3")
            SM = psum.tile([P, 160], f32, name="SM")
            h_ps = SM[0:64, 0:1]
            zg_ps = SM[0:64, 1:2]
            zb_ps = SM[0:64, 2:3]
            dg_ps = SM[:, 3:4]
            db_ps = SM[:, 4:5]
            mu_ps = SM[0:64, 5:6]
            msq_ps = SM[0:64, 6:7]
            mu128_ps = SM[:, 7:8]
            msq128_ps = SM[:, 8:9]
            consts_ps = SM[0:1, 9:12]
            cb9_ps = SM[:, 64:73]
            cb52_ps = SM[:, 73:73 + 4 * (M + 1)]

            # ---------------- SBUF ------------------------------------------
            xpl_a = sb.tile([P, 16 * D], bf16)     # x2 naturals (bf16, host)
            xpl_b = sb.tile([P, 16 * D], bf16)
            xi_a = sb.tile([P, 32 * D], bf16)   # interleaved [x2|x2^2] pairs
            xi_b = sb.tile([P, 32 * D], bf16)
            xs2 = sb.tile([P, S], bf16)            # stacked T-form [x2T; x2sqT]
            x1sbn = sb.tile([P, SSH // 2], bf16)
            x1sqn = sb.tile([P, SSH // 2], bf16)
            x1fbig = sb.tile([P, (S // P) * D], bf16)
            wcat_sb = sb.tile([D, 270], f32)
            lhsT1 = sb.tile([P, 6], bf16)
            lhsTq = sb.tile([P, 4], bf16)
            ident = sb.tile([P, P], bf16)
            identf = sb.tile([D, D], f32)
            ones_col = sb.tile([P, 1], f32)
            ones128 = sb.tile([P, P], f32)
            ones_bf = sb.tile([P, 1], bf16)
            ones_row = sb.tile([1, P], f32)
            eps_col = sb.tile([P, 1], f32)
            mu_sb = sb.tile([D, 1], f32)
            msq_sb = sb.tile([D, 1], f32)
            musq = sb.tile([P, 1], f32)
            mu128s = sb.tile([P, 1], f32)
            var128 = sb.tile([P, 1], f32)
            sq128 = sb.tile([P, 1], f32)
            rs128 = sb.tile([P, 1], f32)
            A128 = sb.tile([P, 1], f32)
            muA = sb.tile([P, 1], f32)
            B128 = sb.tile([P, 1], f32)
            zg_sb = sb.tile([D, 1], f32)
            zb_sb = sb.tile([D, 1], f32)
            h_col = sb.tile([D, 1], f32)
            crow = sb.tile([1, 9], f32)
            constsb = sb.tile([P, 9], f32)
            PPs = sb.tile([P, 1024], bf16)
            TQTs = sb.tile([P, 512], bf16)
            PQs3 = sb.tile([4, 512], bf16)
            rsnn = sb.tile([P, 64], f32)
            stnn = sb.tile([P, 64], f32)
            rsq1 = sb.tile([P, 4], f32)
            st1 = sb.tile([P, 4], f32)
            K_all = sb.tile([P, (M + 1) * 2 * 64], bf16)
            R = sb.tile([P, 4 * (M + 1) + 4], f32)
            cbrow = sb.tile([P, 4 * (M + 1)], f32)
            qhat = sb.tile([P, 8], f32)
            Qs = sb.tile([P, 8], f32)
            eD = sb.tile([P, 8], f32)
            oD = sb.tile([P, 8], f32)
            eN = sb.tile([P, 8], f32)
            oN = sb.tile([P, 8], f32)
            rden = sb.tile([P, 8], f32)
            rr = sb.tile([P, 8], f32)
            zt = sb.tile([P, 8], f32)
            sig = sb.tile([P, 8], f32)

            x2r = x2.rearrange("(p cc) d -> p (cc d)", p=P)
            x1sr = x1s.rearrange("(p cc) d -> p (cc d)", p=P)
            x1fr = x1f.rearrange("(p c) d -> p (c d)", p=P)

            # ===== loads, ordered by when each gating chain needs them ======
            # x2 first (it gates the longest chain: convert/square/transpose/
            # copies); x1f second-to-last (its h->MLP->dg chain is short);
            # x1s last (q path has the most slack).
            nc.sync.dma_start(xpl_a[:], x2r[:, 0:16 * D])
            nc.sync.dma_start(xpl_b[:, 0:8 * D], x2r[:, 16 * D:24 * D])
            nc.sync.dma_start(xpl_b[:, 8 * D:16 * D], x2r[:, 24 * D:32 * D])
            nc.sync.dma_start(wcat_sb[:], wcat[:, :])
            nc.sync.dma_start(x1fbig[:, 0:16 * D], x1fr[:, 0:16 * D])
            nc.sync.dma_start(x1fbig[:, 16 * D:32 * D], x1fr[:, 16 * D:32 * D])
            nc.sync.dma_start(x1sbn[:], x1sr[:, :])

            # static prep
            nc.vector.memset(ones_col[:], 1.0)
            nc.vector.memset(ones128[:], 1.0)
            nc.vector.memset(ones_bf[:], 1.0)
            nc.vector.memset(ones_row[:], 1.0)
            nc.vector.memset(eps_col[:], EPS_BN)
            make_identity(nc, ident[:])
            make_identity(nc, identf[:])
            nc.gpsimd.memset(lhsTq[:], 0.0)
            nc.gpsimd.memset(lhsT1[:], 0.0)
            nc.gpsimd.memset(lhsTq[64:128, 2:3], 1.0)
            nc.gpsimd.memset(lhsT1[64:128, 5:6], 1.0)

            # small-weight casts from the concatenated block (f32 -> bf16)
            nc.vector.tensor_copy(lhsTq[0:64, 0:2], wcat_sb[:, 256:258])  # Wq
            nc.vector.tensor_copy(lhsT1[0:64, 0:2], wcat_sb[:, 258:260])  # Wk
            wvv = wcat_sb[:, 260:262]
            nc.gpsimd.tensor_copy(crow[:, 3:7], wcat_sb[0:1, 262:266])    # Wo
            nc.gpsimd.tensor_copy(crow[:, 7:9], wcat_sb[0:1, 266:268])    # bo

            def kslice(m):
                return K_all[:, (2 * m) * 64:(2 * m + 1) * 64]

            def uslice(m):
                return K_all[:, (2 * m + 1) * 64:(2 * m + 2) * 64]

            # S_0 per-partition partial = 32 (handled as a constant in R)
            nc.vector.memset(R[:, 0:2], 32.0)

            # PE p-state warmup + gap fillers: the tensor engine only reaches
            # full clock after ~3us of continuous execution, so keep it
            # spinning on junk transposes whenever it would otherwise idle.
            # All spins are program-ordered before the real TPn writes (WAW),
            # and the scheduler's priority heap prefers real work when ready.
            for w in range(44):
                nc.tensor.transpose(TP[:, 128 * (w % 8):128 * (w % 8) + 128],
                                    ident[:], ident[:])

            # ============ x2 converts + squares + transposes ================
            # quarter q covers natural chunks cc in [8q, 8q+8); transpose of
            # chunk cc lands at TR[0:64 | 64:128, 128j:128(j+1)], j = cc%8;
            # xs2 column i = 1024q + 128j + p' maps to t = 32p' + cc.
            # Converts (ACT for b) and squares (DVE, straight from the f32
            # naturals) run in parallel; xs2 copy-outs are emitted later in
            # per-engine readiness order.
            nc.vector.memset(PP1[:], 0.0)
            nc.vector.memset(PP2[:], 0.0)
            with tc.high_priority():
                # interleave the host-cast bf16 naturals with their squares
                # (plain copy on ACT, squares on DVE, in parallel)
                xav = xi_a[:].rearrange("p (c two d) -> p c two d", two=2, d=D)
                xbv = xi_b[:].rearrange("p (c two d) -> p c two d", two=2, d=D)
                xanat = xpl_a[:].rearrange("p (c d) -> p c d", d=D)
                xbnat = xpl_b[:].rearrange("p (c d) -> p c d", d=D)
                nc.scalar.copy(xav[:, :, 0, :], xanat)
                nc.vector.tensor_tensor(out=xav[:, :, 1, :], in0=xanat,
                                        in1=xanat, op=ALU.mult)
                for qq in range(2):
                    ql = slice(8 * qq, 8 * qq + 8)
                    nc.scalar.copy(xbv[:, ql, 0, :], xbnat[:, ql, :])
                    nc.vector.tensor_tensor(out=xbv[:, ql, 1, :],
                                            in0=xbnat[:, ql, :],
                                            in1=xbnat[:, ql, :], op=ALU.mult)
                for q in range(4):
                    TR = (TR1, TR2, TR1, TR2)[q]
                    xi = (xi_a, xi_b)[q // 2]
                    for j in range(8):
                        cl = slice((8 * (q % 2) + j) * 2 * D, (8 * (q % 2) + j + 1) * 2 * D)
                        nc.tensor.transpose(TR[:, 128 * j:128 * (j + 1)], xi[:, cl], ident[:])
                    if q == 2:
                        nc.scalar.copy(xs2[:, 1024 * q:1024 * (q + 1)], TR[:])
                    else:
                        nc.vector.tensor_copy(xs2[:, 1024 * q:1024 * (q + 1)], TR[:])
                # per-half stats (ap-1 matmuls are ~free on PE)
                for half, xn in enumerate((xpl_a, xpl_b)):
                    for cc in range(16):
                        nc.tensor.matmul(mu_ps, xn[:, cc * D:(cc + 1) * D], ones_bf[:],
                                         start=(half == 0 and cc == 0), stop=(half == 1 and cc == 15))
                for half, xi in enumerate((xi_a, xi_b)):
                    for cc in range(16):
                        nc.tensor.matmul(msq_ps, xi[:, (2 * cc + 1) * D:(2 * cc + 2) * D], ones_bf[:],
                                         start=(half == 0 and cc == 0), stop=(half == 1 and cc == 15))
                TCf = S // P
                for c in range(TCf):
                    nc.tensor.matmul(h_ps, x1fbig[:, c * D:(c + 1) * D], ones_bf[:],
                                     start=(c == 0), stop=(c == TCf - 1))

            # ============ h epilogue + CBN MLPs (ready before b lands) ======
            nc.scalar.activation(h_col[:], h_ps, AF.Copy, scale=1.0 / S)
            nc.tensor.matmul(zg_ps, wcat_sb[:, 0:64], h_col[:], start=True, stop=True)
            nc.tensor.matmul(zb_ps, wcat_sb[:, 128:192], h_col[:], start=True, stop=True)
            nc.scalar.activation(zg_sb[:], zg_ps, AF.Relu)
            nc.scalar.activation(zb_sb[:], zb_ps, AF.Relu)
            nc.tensor.matmul(dg_ps[0:64, :], wcat_sb[:, 64:128], zg_sb[:], start=True, stop=True)
            nc.tensor.matmul(db_ps[0:64, :], wcat_sb[:, 192:256], zb_sb[:], start=True, stop=True)

            # ============ x1 query path (on-chip transposes) ================
            # slab row s = 4p + cc ; TQT2 col j = 128c + p' <-> s = 4p' + c;
            # partitions 0:64 features, 64:128 squares.
            nc.gpsimd.tensor_tensor(out=x1sqn[:], in0=x1sbn[:], in1=x1sbn[:],
                                    op=ALU.mult)
            for c in range(4):
                nc.tensor.transpose(TQT2[0:64, 128 * c:128 * (c + 1)],
                                    x1sbn[:, 64 * c:64 * (c + 1)], ident[:])
                nc.tensor.transpose(TQT2[64:128, 128 * c:128 * (c + 1)],
                                    x1sqn[:, 64 * c:64 * (c + 1)], ident[:])

            # ============ x2 stats epilogue + A, B, lhsT columns ============
            # single-hop chain on 64 partitions, reading the stat PSUM cells
            # directly (1/S factors folded into the op scalars); only the
            # sqrt visits ACT, and only the A^2 column needs an upper-half
            # duplicate (one identity matmul).
            Aup_ps = mu128_ps[64:128, :]
            nc.scalar.activation(musq[0:64, :], mu_ps, AF.Square, scale=1.0 / S)
            nc.vector.scalar_tensor_tensor(out=var128[0:64, :], in0=msq_ps,
                                           scalar=1.0 / S, in1=musq[0:64, :],
                                           op0=ALU.mult, op1=ALU.subtract)
            nc.scalar.activation(sq128[0:64, :], var128[0:64, :], AF.Sqrt,
                                 bias=eps_col[0:64, :])
            nc.vector.reciprocal(rs128[0:64, :], sq128[0:64, :])
            nc.vector.scalar_tensor_tensor(out=A128[0:64, :], in0=dg_ps[0:64, :],
                                           scalar=1.0, in1=rs128[0:64, :],
                                           op0=ALU.add, op1=ALU.mult)
            nc.vector.scalar_tensor_tensor(out=muA[0:64, :], in0=mu_ps,
                                           scalar=1.0 / S, in1=A128[0:64, :],
                                           op0=ALU.mult, op1=ALU.mult)
            nc.vector.tensor_tensor(out=B128[0:64, :], in0=db_ps[0:64, :],
                                    in1=muA[0:64, :], op=ALU.subtract)
            for hh in range(H):
                nc.vector.tensor_tensor(out=lhsT1[0:64, 2 + hh:3 + hh],
                                        in0=A128[0:64, :], in1=wvv[:, hh:hh + 1], op=ALU.mult)
            nc.vector.scalar_tensor_tensor(out=lhsT1[0:64, 4:5], in0=A128[0:64, :], scalar=2.0,
                                           in1=B128[0:64, :], op0=ALU.mult, op1=ALU.mult)
            nc.tensor.matmul(Aup_ps, identf[:], A128[0:64, :], start=True, stop=True)
            nc.scalar.activation(lhsT1[64:128, 4:5], Aup_ps, AF.Square)

            nc.tensor.matmul(consts_ps[:, 0:2], B128[0:64, :], wvv[:, :], start=True, stop=True)
            nc.tensor.matmul(consts_ps[:, 2:3], B128[0:64, :], B128[0:64, :], start=True, stop=True)
            nc.scalar.copy(crow[:, 0:3], consts_ps)
            nc.tensor.matmul(cb9_ps, ones_row[:], crow[:], start=True, stop=True)
            nc.scalar.copy(constsb[:], cb9_ps)

            # q-projection: one stacked matmul gives (q0, q1, ||q||^2) per s
            # (held out of the heap until the lhsT1 chain has cleared ACT/DVE)
            with tc.tile_wait_until(0.0118):
                nc.scalar.copy(TQTs[:], TQT2[:])
                nc.tensor.matmul(PQ3[0:3, :], lhsTq[:, 0:3], TQTs[:],
                                 start=True, stop=True)
                nc.scalar.copy(PQs3[0:3, :], PQ3[0:3, :])
                for b in range(4):
                    nc.tensor.transpose(TQ[:, 4 * b:4 * b + 3],
                                        PQs3[0:3, 128 * b:128 * (b + 1)],
                                        ident[0:3, 0:3])
            TQv = TQ[:].rearrange("p (b q) -> p b q", q=4)

            # ============ stacked projections ===============================
            # chunk c = 4T + u -> rows 32u..32u+6 of PP{T+1}; after the 4th
            # chunk each bank is copied out whole (rows 6..31 of each 32-row
            # group are pre-zeroed by the memsets above).
            CW = 512
            for c in range(8):
                cs = slice(c * CW, (c + 1) * CW)
                pp = (PP1, PP2)[c // 4]
                u = c % 4
                nc.tensor.matmul(pp[32 * u:32 * u + 6, :], lhsT1[:], xs2[:, cs],
                                 start=True, stop=True, tile_position=(0, 32 * u))
            nc.vector.tensor_copy(PPs[:, 0:512], PP1[:])
            nc.scalar.copy(PPs[:, 512:1024], PP2[:])

            # ============ de-transpose (downstream reads PSUM directly) =====
            for g in range(8):
                nc.tensor.transpose(TP[:, 128 * g:128 * (g + 1)],
                                    PPs[:, 128 * g:128 * (g + 1)], ident[:])
            Cv = TP[:].rearrange("p (g u q) -> p g u q", g=8, u=4)

            # ==== rsqrt = reciprocal(sqrt(x)): ACT sqrt + DVE recip; n2 first
            # (khat needs only rsn2 — keep its chain free of the nv2 sqrt) ==
            stnnv = stnn[:].rearrange("p (g u e) -> p g u e", g=8, u=4)
            rsnnv = rsnn[:].rearrange("p (g u e) -> p g u e", g=8, u=4)
            nc.scalar.activation(stnnv[:, :, :, 1:2], Cv[:, :, :, 5:6], AF.Sqrt)
            nc.vector.reciprocal(rsnnv[:, :, :, 1:2], stnnv[:, :, :, 1:2])
            nc.scalar.activation(stnnv[:, :, :, 0:1], Cv[:, :, :, 4:5], AF.Sqrt,
                                 bias=constsb[:, 2:3])
            nc.vector.reciprocal(rsnnv[:, :, :, 0:1], stnnv[:, :, :, 0:1])
            nc.scalar.activation(st1[:].rearrange("p (b o) -> p b o", o=1),
                                 TQv[:, :, 2:3], AF.Sqrt)
            nc.vector.reciprocal(rsq1[:], st1[:])

            # ============ k^, v^, q^, fused power sums ======================
            # per-head 32-col slices; accum_out collects the per-partition
            # partial power sums directly into R (no trailing reduces)
            khat = kslice(1)
            vhat = uslice(0)
            rsnnv = rsnn[:].rearrange("p (g u e) -> p g u e", g=8, u=4)
            rsn2v = rsnnv[:, :, :, 1:2]
            rsnvv = rsnnv[:, :, :, 0:1]

            def hsl(sl, hh):
                return sl[:, 32 * hh:32 * (hh + 1)].rearrange(
                    "p (g u o) -> p g u o", g=8, o=1)

            # khat + k-power tree on DVE (kp3, kp4 both branch off kp2; the
            # 1/m! factors are folded into the tree scalars); vhat + u1..u3
            # products on Pool; u4 on DVE right after kp4.
            for hh in range(H):
                nc.vector.scalar_tensor_tensor(
                    out=hsl(khat, hh), in0=Cv[:, :, :, hh:hh + 1], scalar=1.0,
                    in1=rsn2v, op0=ALU.mult, op1=ALU.mult,
                    accum_out=R[:, 4 + hh:5 + hh])
            for hh in range(H):  # vhat reads PSUM -> must be DVE, not Pool
                nc.vector.scalar_tensor_tensor(
                    out=hsl(vhat, hh),
                    in0=Cv[:, :, :, 2 + hh:3 + hh], scalar=constsb[:, hh:hh + 1],
                    in1=rsnvv, op0=ALU.add, op1=ALU.mult,
                    accum_out=R[:, 2 + hh:3 + hh])
            for hh in range(H):  # kp2 = khat^2 / 2
                nc.vector.scalar_tensor_tensor(
                    out=hsl(kslice(2), hh), in0=hsl(khat, hh),
                    scalar=0.5, in1=hsl(khat, hh),
                    op0=ALU.mult, op1=ALU.mult,
                    accum_out=R[:, 8 + hh:9 + hh])
            for hh in range(H):  # kp3 = kp2 * khat / 3
                nc.vector.scalar_tensor_tensor(
                    out=hsl(kslice(3), hh), in0=hsl(khat, hh),
                    scalar=1.0 / 3.0, in1=hsl(kslice(2), hh),
                    op0=ALU.mult, op1=ALU.mult,
                    accum_out=R[:, 12 + hh:13 + hh])
            for hh in range(H):  # kp4 = kp2^2 / 6
                nc.vector.scalar_tensor_tensor(
                    out=hsl(kslice(4), hh), in0=hsl(kslice(2), hh),
                    scalar=1.0 / 6.0, in1=hsl(kslice(2), hh),
                    op0=ALU.mult, op1=ALU.mult,
                    accum_out=R[:, 16 + hh:17 + hh])
            # u-products on Pool (its ISA has no TensorScalarPtr/accum);
            # one strided DVE reduce collects all eight partial sums
            for m in range(1, M + 1):
                for hh in range(H):
                    nc.gpsimd.tensor_tensor(
                        out=hsl(uslice(m), hh), in0=hsl(kslice(m), hh),
                        in1=hsl(vhat, hh), op=ALU.mult)
            Kv = K_all[:].rearrange("p (m two h x) -> p m two h x",
                                    two=2, h=2, x=32)
            Ruv = R[:, 6:22].rearrange("p (m o q) -> p m o q", m=4, o=1)
            nc.vector.reduce_sum(Ruv[:, :, :, 0:2], Kv[:, 1:5, 1:2, :, :],
                                 axis=mybir.AxisListType.X)
            rsq1v = rsq1[:].rearrange("p (b o) -> p b o", o=1)
            for hh in range(H):
                nc.vector.tensor_tensor(
                    out=qhat[:, 4 * hh:4 * (hh + 1)].rearrange("p (b o) -> p b o", o=1),
                    in0=TQv[:, :, hh:hh + 1], in1=rsq1v, op=ALU.mult)
            # all-ones lhsT: one matmul = column sums replicated on all partitions
            nc.tensor.matmul(cb52_ps, ones128[:], R[:], start=True, stop=True)
            nc.vector.tensor_copy(cbrow[:], cb52_ps)

            # ===== Horner, even/odd split (den on DVE, num on Pool) =========
            # p(q) = E(Q) + q*O(Q), Q=q^2; E=c0+c2*Q+c4*Q^2, O=c1+c3*Q
            qx = qhat[:].rearrange("p (h c) -> p h c", h=2)
            Qv = Qs[:].rearrange("p (h c) -> p h c", h=2)

            def cb(i):
                return (cbrow[:, i:i + 2]
                        .rearrange("p (h o) -> p h o", h=2).to_broadcast((P, 2, 4)))

            def cbden(m):
                return cb(4 * m)

            def cbnum(m):
                return cb(4 * m + 2)

            eDv = eD[:].rearrange("p (h c) -> p h c", h=2)
            oDv = oD[:].rearrange("p (h c) -> p h c", h=2)
            eNv = eN[:].rearrange("p (h c) -> p h c", h=2)
            oNv = oN[:].rearrange("p (h c) -> p h c", h=2)
            nc.vector.tensor_tensor(out=Qv, in0=qx, in1=qx, op=ALU.mult)
            # den (DVE)
            nc.vector.tensor_tensor(out=eDv, in0=Qv, in1=cbden(4), op=ALU.mult)
            nc.vector.tensor_tensor(out=oDv, in0=Qv, in1=cbden(3), op=ALU.mult)
            nc.vector.tensor_tensor(out=eDv, in0=eDv, in1=cbden(2), op=ALU.add)
            nc.vector.tensor_tensor(out=oDv, in0=oDv, in1=cbden(1), op=ALU.add)
            nc.vector.tensor_tensor(out=eDv, in0=eDv, in1=Qv, op=ALU.mult)
            nc.vector.tensor_tensor(out=oDv, in0=oDv, in1=qx, op=ALU.mult)
            nc.vector.tensor_tensor(out=eDv, in0=eDv, in1=cbden(0), op=ALU.add)
            nc.vector.tensor_tensor(out=eDv, in0=eDv, in1=oDv, op=ALU.add)
            # num (Pool)
            nc.gpsimd.tensor_tensor(out=eNv, in0=Qv, in1=cbnum(4), op=ALU.mult)
            nc.gpsimd.tensor_tensor(out=oNv, in0=Qv, in1=cbnum(3), op=ALU.mult)
            nc.gpsimd.tensor_tensor(out=eNv, in0=eNv, in1=cbnum(2), op=ALU.add)
            nc.gpsimd.tensor_tensor(out=oNv, in0=oNv, in1=cbnum(1), op=ALU.add)
            nc.gpsimd.tensor_tensor(out=eNv, in0=eNv, in1=Qv, op=ALU.mult)
            nc.gpsimd.tensor_tensor(out=oNv, in0=oNv, in1=qx, op=ALU.mult)
            nc.gpsimd.tensor_tensor(out=eNv, in0=eNv, in1=cbnum(0), op=ALU.add)
            nc.gpsimd.tensor_tensor(out=eNv, in0=eNv, in1=oNv, op=ALU.add)

            # ============ epilogue =========================================
            nc.vector.reciprocal(rden[:], eD[:])
            nc.vector.tensor_tensor(out=rr[:], in0=eN[:], in1=rden[:], op=ALU.mult)
            r0 = rr[:, 0:4].rearrange("p (c o) -> p c o", o=1)
            r1 = rr[:, 4:8].rearrange("p (c o) -> p c o", o=1)
            Zv = zt[:].rearrange("p (c j) -> p c j", j=2)
            nc.vector.tensor_scalar(out=Zv[:, :, 0:1], in0=r0,
                                    scalar1=constsb[:, 3:4], scalar2=constsb[:, 7:8],
                                    op0=ALU.mult, op1=ALU.add)
            nc.vector.tensor_scalar(out=Zv[:, :, 1:2], in0=r0,
                                    scalar1=constsb[:, 4:5], scalar2=constsb[:, 8:9],
                                    op0=ALU.mult, op1=ALU.add)
            nc.vector.scalar_tensor_tensor(out=Zv[:, :, 0:1], in0=r1,
                                           scalar=constsb[:, 5:6],
                                           in1=Zv[:, :, 0:1],
                                           op0=ALU.mult, op1=ALU.add)
            nc.vector.scalar_tensor_tensor(out=Zv[:, :, 1:2], in0=r1,
                                           scalar=constsb[:, 6:7],
                                           in1=Zv[:, :, 1:2],
                                           op0=ALU.mult, op1=ALU.add)
            nc.scalar.activation(sig[:], zt[:], AF.Sigmoid)
            nc.sync.dma_start(y.rearrange("(p c) j -> p (c j)", p=P), sig[:])

    if split:
        _split_waits(nc, mybir)
    return nc


def _split_waits(nc, mybir, maxw=1):
    """This container's walrus build rejects instructions carrying more than
    ~2 sync-wait commands. Split excess waits onto zero-register-write nops
    inserted just before the instruction on the same engine (same-engine
    program order preserves the wait-before-execute semantics)."""
    ctr = 0
    for bb in nc.m.functions[0].blocks:
        new = []
        for inst in bb.instructions:
            si = inst.sync_info
            if si is not None and si.on_wait and len(si.on_wait) > maxw:
                waits = list(si.on_wait)
                ename = str(inst.engine).split(".")[-1]
                for w in waits[:-maxw]:
                    ctr += 1
                    new.append(mybir.InstRegisterMove(
                        name=f"WS-{ctr}",
                        ins=[mybir.ImmediateValue(kind="imm_value", dtype=mybir.dt.int32, value=0)],
                        outs=[mybir.RegisterAccess(kind="register_access", regref=f"{ename}_zero", dtype=mybir.dt.int32)],
                        engine=inst.engine,
                        sync_info=mybir.SyncInfo(on_wait=[w], on_update=[]),
                    ))
                si.on_wait = waits[-maxw:]
            new.append(inst)
        bb.instructions = new
    return nc


def _get_program():
    if "nc" not in _CACHE:
        _CACHE["nc"] = _build()
    return _CACHE["nc"]


def kernel(x1, x2, Wq, Wk, Wv, Wo, bo, Wg1, Wg2, Wb1, Wb2):
    import ml_dtypes
    from concourse import bass_utils

    nc = _get_program()
    bft = ml_dtypes.bfloat16
    x1s_full = np.ascontiguousarray(x1[0]).astype(bft)  # [4096, 64]
    x2s = np.ascontiguousarray(x2[0]).astype(bft)

    wcat = np.zeros((D, 270), dtype=np.float32)
    wcat[:, 0:64] = Wg1
    wcat[:, 64:128] = Wg2
    wcat[:, 128:192] = Wb1
    wcat[:, 192:256] = Wb2
    wcat[:, 256:258] = Wq
    wcat[:, 258:260] = Wk
    wcat[:, 260:262] = Wv
    wcat[0, 262:266] = Wo.reshape(-1)
    wcat[0, 266:268] = bo

    in_maps = []
    for i in range(NCORES):
        in_maps.append({
            "x1s": np.ascontiguousarray(x1s_full[i * SSH:(i + 1) * SSH]),
            "x1f": x1s_full,
            "x2": x2s,
            "wcat": wcat,
        })

    # First execution of a freshly-compiled NEFF occasionally reports a
    # transient device error through the PJRT proxy; a retry succeeds.
    last_err = None
    for attempt in range(3):
        try:
            res = bass_utils.run_bass_kernel_spmd(nc, in_maps, core_ids=list(range(NCORES)))
            out = np.concatenate([res.results[i]["y"] for i in range(NCORES)], axis=0)
            return out.reshape(1, S, 2)
        except Exception as e:  # noqa: BLE001
            last_err = e
            import time
            time.sleep(5)
    raise last_err
